# revision 1
# baseline (speedup 1.0000x reference)
"""Trainium2 Bass kernel for batched TreeCRF message passing.

Reference semantics (per depth layer d):
    x[b,c,w]   = emissions[b,c,layer[w]] + messages[b,c,layer[w]]
    elem[n,b,k,w] = logsumexp_c(x[b,c,w] + transitions[n, layer[w], k, c])
    messages[b,k,n] += sum_w elem[n,b,k,w] * succ[d,w,n]

The successor matrix rows are one-hot (<=1 nonzero per source node, targets in
the next layer, last layer all-zero).  For such S the dense [N,B,K,W] elem
tensor is redundant: only elem[tgt_w, :, :, w] survives the einsum, and since
the einsum is linear in S, gathering the single 2x2 transition block per source
node and scattering through the 100x100 block of S (actual values, collisions
summed by a matmul) is mathematically exact.

Host side therefore only *gathers/reorders* inputs (no arithmetic); the device
kernel performs every add / max / exp / ln / matmul:
  - layout: tree width W=100 on SBUF partitions, (depth, class, batch) on the
    free axis; messages stay resident in SBUF across all 9 scan steps
  - per step: x = em + msg; s_k = x + t_k; stable 2-way logsumexp via
    m + ln(exp(min-max) + 1); scatter = matmul(succ_block^T-free, elem)
  - batch is sharded 8-way across NeuronCores (8 batch elems per core);
    transitions/successor blocks are replicated; no cross-core communication.

If the one-hot structure does not hold, falls back to a faithful numpy
implementation of the reference.
"""

import numpy as np

BATCH, C, N_LABELS = 64, 2, 1000
DEPTH, WIDTH = 10, 100
N_CORES = 8
B = BATCH // N_CORES  # batch elements per core
KB = C * B

_BASS_CACHE = {}


def _no_barrier_block(nc, bass):
    """Like nc.Block() but skips the exit drain + all-engine barrier (~6us of
    event-semaphore ping-pong after the output DMA has already been waited
    on explicitly)."""
    from contextlib import contextmanager

    class _NBBlock(bass.BassBlock):
        def __exit__(self, exc_type, exc_val, exc_tb):
            if exc_type is None:
                for engine, last_body in self.last_body.items():
                    with self.bass.body(
                        last_body,
                        parent=self.bass.cur_bb,
                        allow_existing_parent=True,
                    ):
                        engine.br(self.end_bb)
                self.bass.switch_bb(self.end_bb)

    @contextmanager
    def _ctx():
        assert nc.cur_block is None
        with _NBBlock(nc, f"block_{nc.next_id()}") as blk:
            nc.cur_block = blk
            yield blk
        nc.cur_block = None

    return _ctx()


def _build_bass(use_bcast=True):
    import concourse.bass as bass
    import concourse.mybir as mybir

    W, D = WIDTH, DEPTH
    F32 = mybir.dt.float32

    nc = bass.Bass()
    # emissions and gathered transitions packed per depth layer; the first
    # two layers travel in a small early DMA so compute can start while the
    # bulk transfer's completion semaphore is still in flight
    PK = C * B + C * C * B  # 48 floats per (w, d)
    ett = nc.declare_dram_parameter("ett", [W, D, PK], F32, isOutput=False)
    sbk = nc.declare_dram_parameter("sbk", [128, D - 1, W], F32, isOutput=False)
    out = nc.declare_dram_parameter("out", [W, D, C, B], F32, isOutput=True)

    from contextlib import ExitStack

    with ExitStack() as _es:
        ett_s = _es.enter_context(nc.sbuf_tensor("ett_s", [W, D, PK], F32))
        sb_s = _es.enter_context(nc.sbuf_tensor("sb_s", [128, D - 1, W], F32))
        msg_s = _es.enter_context(nc.sbuf_tensor("msg_s", [W, D, C, B], F32))
        base_s = _es.enter_context(nc.sbuf_tensor("base_s", [W, D, C, C, B], F32))
        s_s = _es.enter_context(nc.sbuf_tensor("s_s", [W, C, C, B], F32))
        mx_s = _es.enter_context(nc.sbuf_tensor("mx_s", [W, C, B], F32))
        mn_s = _es.enter_context(nc.sbuf_tensor("mn_s", [W, C, B], F32))
        sp_s = _es.enter_context(nc.sbuf_tensor("sp_s", [W, C, B], F32))
        elem_s = _es.enter_context(nc.sbuf_tensor("elem_s", [128, KB], F32))
        ptA = _es.enter_context(nc.psum_tensor("ptA", [128, KB], F32))
        ptB = _es.enter_context(nc.psum_tensor("ptB", [128, KB], F32))
        dma_ett = _es.enter_context(nc.semaphore("dma_ett"))
        dma_ett2 = _es.enter_context(nc.semaphore("dma_ett2"))
        dma_sbk = _es.enter_context(nc.semaphore("dma_sbk"))
        dma_out = _es.enter_context(nc.semaphore("dma_out"))
        v2s = _es.enter_context(nc.semaphore("v2s"))
        s2v = _es.enter_context(nc.semaphore("s2v"))
        v2p = _es.enter_context(nc.semaphore("v2p"))
        p2v = _es.enter_context(nc.semaphore("p2v"))
        vdone = _es.enter_context(nc.semaphore("vdone"))
        block = _es.enter_context(_no_barrier_block(nc, bass))
        pts = [ptA, ptB]
        def em_v(d):
            return ett_s[:, d, : C * B].rearrange("w (c b) -> w c b", b=B)

        def tt_v(d):
            return ett_s[:, d, C * B :].rearrange("w (k c b) -> w k c b", c=C, b=B)

        def pt_view(d):
            # matmul output of step d, as (w, c, b), broadcast over next k
            p = pts[d % 2][:W, :].rearrange("w (c b) -> w c b", b=B)
            return p[:, None, :, :].to_broadcast([W, C, C, B])

        def base(vector, d):
            # base_d = emissions (k-broadcast) + gathered transitions
            if use_bcast:
                vector.tensor_add(
                    base_s[:, d],
                    em_v(d)[:, None, :, :].to_broadcast([W, C, C, B]),
                    tt_v(d),
                )
            else:
                vector.tensor_add(base_s[:, d, 0], em_v(d), tt_v(d)[:, 0])
                vector.tensor_add(base_s[:, d, 1], em_v(d), tt_v(d)[:, 1])

        @block.sync
        def _(sync):
            sync.dma_start(ett_s[:, 2:], ett[:, 2:]).then_inc(dma_ett2, 16)

        @block.gpsimd
        def _(gpsimd):
            # first two layers ride an otherwise-empty queue so their
            # completion semaphore isn't stuck behind the bulk transfer
            gpsimd.dma_start(ett_s[:, :2], ett[:, :2]).then_inc(dma_ett, 16)

        @block.vector
        def _(vector):
            # drain() between dependent same-engine ops: the compute pipes do
            # not interlock RAW within an engine, so a consumer issued right
            # after its producer streams stale SBUF.
            vector.memset(msg_s[:], 0.0)
            vector.memset(elem_s[:], 0.0)
            vector.wait_ge(dma_ett, 16)
            base(vector, 0)
            vector.drain()
            for d in range(D - 1):
                has_base = d + 2 <= D - 2
                if d == 0:
                    src = base_s[:, 0]  # messages are zero at the first layer
                else:
                    vector.wait_ge(p2v, d)
                    vector.tensor_add(s_s[:], base_s[:, d], pt_view(d - 1))
                    # an independent op doubles as the pipe gap before s_s is
                    # read: prefer next-next base (keeps the msg copy off the
                    # critical chain), fall back to the msg copy
                    if has_base:
                        base(vector, d + 2)
                    else:
                        vector.tensor_copy(
                            msg_s[:, d],
                            pts[(d - 1) % 2][:W, :].rearrange("w (c b) -> w c b", b=B),
                        )
                    src = s_s[:]
                # stable 2-way logsumexp over c: max + ln(exp(-|s0-s1|)+1).
                # diff first; the max op doubles as the pipe gap before the
                # in-place delta = min(-diff, diff) = -|diff|.
                vector.tensor_sub(mn_s[:], src[:, :, 0, :], src[:, :, 1, :])
                vector.tensor_tensor(
                    mx_s[:], src[:, :, 0, :], src[:, :, 1, :], mybir.AluOpType.max
                )
                vector.scalar_tensor_tensor(
                    mn_s[:], mn_s[:], -1.0, mn_s[:],
                    mybir.AluOpType.mult, mybir.AluOpType.min,
                ).then_inc(v2s, 1)
                # slack work while the scalar engine runs exp/ln
                if d == 0 and has_base:
                    base(vector, 1)
                    vector.wait_ge(dma_ett2, 16)
                    base(vector, d + 2)
                elif d >= 1 and has_base:
                    vector.tensor_copy(
                        msg_s[:, d],
                        pts[(d - 1) % 2][:W, :].rearrange("w (c b) -> w c b", b=B),
                    )
                vector.wait_ge(s2v, d + 1)
                vector.tensor_add(
                    elem_s[:W, :].rearrange("w (k b) -> w k b", b=B), mx_s[:], sp_s[:]
                ).then_inc(v2p, 1)
            vector.wait_ge(p2v, D - 1)
            vector.tensor_copy(
                msg_s[:, D - 1],
                pts[(D - 2) % 2][:W, :].rearrange("w (c b) -> w c b", b=B),
            ).then_inc(vdone, 1)


        @block.scalar
        def _(scalar):
            scalar.dma_start(sb_s[:], sbk[:]).then_inc(dma_sbk, 16)
            # dummy activation pulls the exp/ln ACT table load off the
            # critical path (overlaps the input DMAs)
            scalar.activation(
                sp_s[:1, :1, :1], sp_s[:1, :1, :1], mybir.ActivationFunctionType.Exp
            )
            scalar.drain()
            for d in range(D - 1):
                scalar.wait_ge(v2s, d + 1)
                # softplus(delta) = ln(exp(delta) + 1); exp and ln share one
                # ACT table set so there is no table-swap cost.
                scalar.activation(
                    sp_s[:], mn_s[:], mybir.ActivationFunctionType.Exp
                )
                scalar.nop(cycle_cnt=24)
                scalar.activation(
                    sp_s[:], sp_s[:], mybir.ActivationFunctionType.Ln, bias=1.0
                ).then_inc(s2v, 1)
            # layers 0..8 are final once copy(8) landed (guaranteed before
            # this engine saw v2s >= 9); layer 9 comes straight from the last
            # matmul's PSUM bank. No completion wait: the NEFF postamble
            # barrier (~7us) far outlasts these transfers.
            scalar.dma_start(out[:, : D - 1], msg_s[:, : D - 1]).then_inc(dma_out, 16)
            scalar.wait_ge(vdone, 1)
            scalar.dma_start(out[:, D - 1], msg_s[:, D - 1]).then_inc(dma_out, 16)

        @block.tensor
        def _(tensor):
            tensor.wait_ge(dma_sbk, 16)
            for d in range(D - 1):
                tensor.wait_ge(v2p, d + 1)
                # contract over only the 100 live rows (smaller LDWEIGHTS)
                tensor.matmul(
                    pts[d % 2][:W, :], sb_s[:W, d], elem_s[:W, :],
                    start=True, stop=True,
                ).then_inc(p2v, 1)

    return nc


def _fast_path_ok(emissions, transitions, layer_ids, succ):
    if emissions.shape != (BATCH, C, N_LABELS):
        return False
    if transitions.shape != (N_LABELS, N_LABELS, C, C):
        return False
    if layer_ids.shape != (DEPTH, WIDTH) or succ.shape != (DEPTH, WIDTH, N_LABELS):
        return False
    # layer_ids must cover every label exactly once
    if not np.array_equal(np.sort(layer_ids.reshape(-1)), np.arange(N_LABELS)):
        return False
    nz = succ != 0
    # each source node sends at most one message
    if nz.sum(axis=-1).max(initial=0) > 1:
        return False
    # last layer has no successors
    if nz[DEPTH - 1].any():
        return False
    # all targets live in the next layer
    for d in range(DEPTH - 1):
        in_block = nz[d][:, layer_ids[d + 1]].sum()
        if in_block != nz[d].sum():
            return False
    return True


def _numpy_fallback(emissions, transitions, layer_ids, succ):
    messages = np.zeros_like(emissions)
    for d in range(layer_ids.shape[0]):
        layer = layer_ids[d]
        S = succ[d]
        x = emissions[:, :, layer] + messages[:, :, layer]          # [B,C,W]
        t = np.transpose(transitions[:, layer], (0, 2, 3, 1))       # [N,K,C,W]
        z = x[None, :, None, :, :] + t[:, None, :, :, :]            # [N,B,K,C,W]
        m = z.max(axis=3, keepdims=True)
        elem = np.squeeze(m, 3) + np.log(np.exp(z - m).sum(axis=3))
        messages = messages + np.einsum("nbkw,wn->bkn", elem, S)
    return messages


def kernel(emissions, transitions, layer_ids, succ):
    from concourse.bass_utils import run_bass_kernel_spmd

    emissions = np.ascontiguousarray(np.asarray(emissions, dtype=np.float32))
    transitions = np.ascontiguousarray(np.asarray(transitions, dtype=np.float32))
    layer_ids = np.asarray(layer_ids).astype(np.int64)
    succ = np.ascontiguousarray(np.asarray(succ, dtype=np.float32))

    if not _fast_path_ok(emissions, transitions, layer_ids, succ):
        return _numpy_fallback(emissions, transitions, layer_ids, succ)

    nz = succ != 0
    tgt = np.argmax(nz, axis=-1)  # [D, W]; 0 for empty rows (value unused: S=0)

    # gathered 2x2 transition block per source node, replicated across batch:
    # ttp[w, d, k, c, b] = transitions[tgt[d,w], layer_ids[d,w], k, c]
    tt = transitions[tgt, layer_ids]                 # [D, W, C, C]
    ttp = np.broadcast_to(tt[..., None], (DEPTH, WIDTH, C, C, B))
    ttp = np.ascontiguousarray(ttp.transpose(1, 0, 2, 3, 4))  # [W, D, C, C, B]

    # successor block restricted to the next layer's labels (lhsT for matmul),
    # padded to 128 partitions for a full-width contraction
    sbk = np.zeros((128, DEPTH - 1, WIDTH), dtype=np.float32)
    for d in range(DEPTH - 1):
        sbk[:WIDTH, d, :] = succ[d][:, layer_ids[d + 1]]

    # per-core emissions gathered into (w, d, c, b) layout, packed per depth
    # layer together with the (shared) gathered transitions
    em_sh = emissions.reshape(N_CORES, B, C, N_LABELS)
    ttp_d = ttp.reshape(WIDTH, DEPTH, C * C * B)
    in_maps = []
    for i in range(N_CORES):
        g = em_sh[i][:, :, layer_ids]                # [B, C, D, W]
        emT = g.transpose(3, 2, 1, 0).reshape(WIDTH, DEPTH, C * B)
        ett = np.ascontiguousarray(np.concatenate([emT, ttp_d], axis=2))
        in_maps.append({"ett": ett, "sbk": sbk})

    if "nc" not in _BASS_CACHE:
        _BASS_CACHE["nc"] = _build_bass()
    res = run_bass_kernel_spmd(_BASS_CACHE["nc"], in_maps, core_ids=list(range(N_CORES)))

    out = np.zeros((BATCH, C, N_LABELS), dtype=np.float32)
    for i in range(N_CORES):
        m = res.results[i]["out"]                    # [W, D, C, B]
        mt = m.transpose(3, 2, 1, 0)                 # [B, C, D, W]
        blk = np.zeros((B, C, N_LABELS), dtype=np.float32)
        blk[:, :, layer_ids] = mt
        out[i * B : (i + 1) * B] = blk
    return out



# revision 14
# speedup vs baseline: 1.2728x; 1.2728x over previous
"""Trainium2 Bass kernel for batched TreeCRF message passing.

Reference semantics (per depth layer d):
    x[b,c,w]   = emissions[b,c,layer[w]] + messages[b,c,layer[w]]
    elem[n,b,k,w] = logsumexp_c(x[b,c,w] + transitions[n, layer[w], k, c])
    messages[b,k,n] += sum_w elem[n,b,k,w] * succ[d,w,n]

The successor matrix rows are one-hot, so only the 2x2 transition block of the
single target node per source survives, and the dense scatter is a 100x100
matmul per layer (collisions summed exactly).  Using the identity
    lse(s0, s1) = s1 + softplus(s0 - s1)
each scan step needs only
    diff = dbase_d + (P0 - P1)          (P = incoming messages, psum)
    sp   = softplus(diff)               (ACT, single table op)
    s1   = base1_d + P1                 (POOL)
    P'   = S_d^T s1 + S_d^T sp          (PE, fp16 weights/rhs, fp32 psum)
where base_d = emissions + transitions gathered per layer (precomputed by the
vector engine in slack time) and dbase_d = base_d[c=0] - base_d[c=1].

Engine layout per step: PE psum -> V (dpt, diff) -> ACT softplus -> PE matmul,
with POOL computing s1 and archiving psum->sbuf output copies in parallel.
Weights and matmul operands travel as fp16 (succ entries 0/1 are exact; s1/sp
quantization is ~5e-4 relative), making every matmul single-pass.

Host side only gathers/reorders inputs (no arithmetic); batch is sharded
8-way across cores; transitions/successor blocks replicated; no cross-core
communication.  Falls back to a faithful numpy implementation if the one-hot
structure does not hold.
"""

import numpy as np

BATCH, C, N_LABELS = 64, 2, 1000
DEPTH, WIDTH = 10, 100
N_CORES = 8
B = BATCH // N_CORES  # batch elements per core
KB = C * B            # 16
D1 = DEPTH - 1        # 9 scan steps / active layers

_BASS_CACHE = {}


def _no_barrier_block(nc, bass):
    """Like nc.Block() but skips the exit drain + all-engine barrier (~6us of
    event-semaphore ping-pong after the output DMA has already been issued)."""
    from contextlib import contextmanager

    class _NBBlock(bass.BassBlock):
        def __exit__(self, exc_type, exc_val, exc_tb):
            if exc_type is None:
                for engine, last_body in self.last_body.items():
                    with self.bass.body(
                        last_body,
                        parent=self.bass.cur_bb,
                        allow_existing_parent=True,
                    ):
                        engine.br(self.end_bb)
                self.bass.switch_bb(self.end_bb)

    @contextmanager
    def _ctx():
        assert nc.cur_block is None
        with _NBBlock(nc, f"block_{nc.next_id()}") as blk:
            nc.cur_block = blk
            yield blk
        nc.cur_block = None

    return _ctx()


def _build_bass():
    import concourse.bass as bass
    import concourse.mybir as mybir

    W = WIDTH
    F32 = mybir.dt.float32
    F16 = mybir.dt.float16
    EXP = mybir.ActivationFunctionType.Exp
    LN = mybir.ActivationFunctionType.Ln

    nc = bass.Bass()
    # ett[w, d, 0:32]  = emissions dup'd over k, layout (c, k, b)
    # ett[w, d, 32:64] = gathered transitions dup'd over b, layout (c, k, b)
    ett = nc.declare_dram_parameter("ett", [W, D1, 64], F32, isOutput=False)
    sbk = nc.declare_dram_parameter("sbk", [W, D1, W], F16, isOutput=False)
    # out[w, j, (c, b)] = messages into layer j+1 (layer 0 receives nothing)
    out = nc.declare_dram_parameter("out", [W, D1, KB], F32, isOutput=True)

    from contextlib import ExitStack

    with ExitStack() as _es:
        ett_s = _es.enter_context(nc.sbuf_tensor("ett_s", [W, D1, 64], F32))
        sbk_s = _es.enter_context(nc.sbuf_tensor("sbk_s", [W, D1, W], F16))
        base_s = _es.enter_context(nc.sbuf_tensor("base_s", [W, D1, 32], F32))
        dbase_s = _es.enter_context(nc.sbuf_tensor("dbase_s", [W, D1, KB], F32))
        diff_s = _es.enter_context(nc.sbuf_tensor("diff_s", [W, 2, KB], F32))
        dpt_s = _es.enter_context(nc.sbuf_tensor("dpt_s", [W, KB], F32))
        sp_s = _es.enter_context(nc.sbuf_tensor("sp_s", [W, 2, KB], F16))
        esp_s = _es.enter_context(nc.sbuf_tensor("esp_s", [W, KB], F32))
        s1_s = _es.enter_context(nc.sbuf_tensor("s1_s", [W, 2, KB], F16))
        s10_s = _es.enter_context(nc.sbuf_tensor("s10_s", [W, KB], F16))
        msg_s = _es.enter_context(nc.sbuf_tensor("msg_s", [W, D1, KB], F32))
        scr_s = _es.enter_context(nc.sbuf_tensor("scr_s", [1, 2], F32))
        ptA = _es.enter_context(nc.psum_tensor("ptA", [128, KB], F32))
        ptB = _es.enter_context(nc.psum_tensor("ptB", [128, KB], F32))
        q_ett1 = _es.enter_context(nc.semaphore("q_ett1"))
        q_ett2 = _es.enter_context(nc.semaphore("q_ett2"))
        q_sbk1 = _es.enter_context(nc.semaphore("q_sbk1"))
        q_sbk2 = _es.enter_context(nc.semaphore("q_sbk2"))
        q_out = _es.enter_context(nc.semaphore("q_out"))
        v2s = _es.enter_context(nc.semaphore("v2s"))
        v2t = _es.enter_context(nc.semaphore("v2t"))
        a2t = _es.enter_context(nc.semaphore("a2t"))
        p2t = _es.enter_context(nc.semaphore("p2t"))
        p2v = _es.enter_context(nc.semaphore("p2v"))
        pl2v = _es.enter_context(nc.semaphore("pl2v"))
        pdone = _es.enter_context(nc.semaphore("pdone"))
        block = _es.enter_context(_no_barrier_block(nc, bass))
        pts = [ptA, ptB]

        def p_lo(d):
            # c=0 half of the step-d psum, k-broadcast to (k, b)
            p = pts[d % 2][:W, :B]
            return p[:, None, :].to_broadcast([W, C, B])

        def p_hi(d):
            p = pts[d % 2][:W, B:]
            return p[:, None, :].to_broadcast([W, C, B])

        def kb(ap):
            return ap.rearrange("w (k b) -> w k b", b=B)

        def base(vector, d):
            # base_d[w, (c,k,b)] = em_dup + tt_dup   (one flat 32-wide add)
            vector.tensor_add(base_s[:, d], ett_s[:, d, :32], ett_s[:, d, 32:])

        def dbase(vector, d):
            return vector.tensor_sub(
                dbase_s[:, d], base_s[:, d, :KB], base_s[:, d, KB:]
            )

        @block.scalar
        def _(scalar):
            # critical first chunk rides the fastest queue; the softplus table
            # load (~1.3us) overlaps the DMA flight
            scalar.dma_start(ett_s[:, :2], ett[:, :2]).then_inc(q_ett1, 16)
            scalar.activation(scr_s[:1, :1], scr_s[:1, 1:], EXP)
            scalar.drain()
            for d in range(D1):
                scalar.wait_ge(v2s, d + 1)
                src = dbase_s[:, 0] if d == 0 else diff_s[:, d % 2]
                # softplus(diff) = ln(exp(diff) + 1); exp/ln share one ACT
                # table so there is no per-step table swap
                scalar.activation(esp_s[:], src, EXP)
                scalar.nop(cycle_cnt=24)
                scalar.activation(sp_s[:, d % 2], esp_s[:], LN, bias=1.0).then_inc(
                    a2t, 1
                )

        @block.sync
        def _(sync):
            sync.dma_start(sbk_s[:, :2], sbk[:, :2]).then_inc(q_sbk1, 16)
            sync.dma_start(sbk_s[:, 2:], sbk[:, 2:]).then_inc(q_sbk2, 16)
            sync.wait_ge(pdone, 1)
            sync.dma_start(out[:], msg_s[:]).then_inc(q_out, 16)

        @block.gpsimd
        def _(gpsimd):
            # Pool cannot touch PSUM; it owns the bulk ett DMA and the
            # SBUF-only base/dbase precompute for layers 2..8
            gpsimd.dma_start(ett_s[:, 2:], ett[:, 2:]).then_inc(q_ett2, 16)
            gpsimd.wait_ge(q_ett2, 16)
            for d in range(2, D1):
                base(gpsimd, d)
            # all dbase reads hit a base written >=4 ops earlier (no RAW gap
            # needed); chunked incs so the vector engine never stalls long
            dbase(gpsimd, 2)
            dbase(gpsimd, 3).then_inc(pl2v, 1)   # steps 2-3
            dbase(gpsimd, 4)
            dbase(gpsimd, 5).then_inc(pl2v, 1)   # steps 4-5
            dbase(gpsimd, 6)
            dbase(gpsimd, 7).then_inc(pl2v, 1)   # steps 6-7
            dbase(gpsimd, 8).then_inc(pl2v, 1)   # step 8

        @block.vector
        def _(vector):
            # NOTE: DVE does not interlock RAW within the engine; dependent
            # ops are separated by an independent op (issue spacing ~95ns,
            # op duration ~190ns).
            vector.wait_ge(q_ett1, 16)
            base(vector, 0)
            base(vector, 1)
            # dbase_0 is step 0's softplus input (RAW gap after base_0 = base_1)
            dbase(vector, 0).then_inc(v2s, 1)
            vector.tensor_copy(s10_s[:], base_s[:, 0, KB:]).then_inc(v2t, 1)
            dbase(vector, 1)
            for d in range(1, D1):
                vector.wait_ge(p2v, d)
                if d >= 2 and d % 2 == 0:
                    vector.wait_ge(pl2v, d // 2)
                # diff = dbase + P0 - P1 in two ops (an instruction may read
                # at most one PSUM operand)
                vector.tensor_add(
                    kb(dpt_s[:]), kb(dbase_s[:, d]), p_lo(d - 1)
                )
                # s1_d = base_d[c=1] + P1 (k-broadcast) -> fp16; doubles as
                # the RAW pipe gap before diff reads the partial sum
                vector.tensor_add(
                    kb(s1_s[:, d % 2]), kb(base_s[:, d, KB:]), p_hi(d - 1)
                ).then_inc(p2t, 1)
                vector.tensor_sub(
                    kb(diff_s[:, d % 2]), kb(dpt_s[:]), p_hi(d - 1)
                ).then_inc(v2s, 1)
                # archive the finished message layer (psum -> sbuf)
                vector.tensor_copy(msg_s[:, d - 1], pts[(d - 1) % 2][:W, :])
            vector.wait_ge(p2v, D1)
            vector.tensor_copy(msg_s[:, D1 - 1], pts[(D1 - 1) % 2][:W, :]).then_inc(
                pdone, 1
            )

        @block.tensor
        def _(tensor):
            tensor.wait_ge(q_sbk1, 16)
            tensor.wait_ge(v2t, 1)
            tensor.matmul(ptA[:W, :], sbk_s[:, 0], s10_s[:], start=True, stop=False)
            tensor.wait_ge(a2t, 1)
            tensor.matmul(
                ptA[:W, :], sbk_s[:, 0], sp_s[:, 0], start=False, stop=True
            ).then_inc(p2v, 1)
            for d in range(1, D1):
                if d == 2:
                    tensor.wait_ge(q_sbk2, 16)
                tensor.wait_ge(p2t, d)
                tensor.matmul(
                    pts[d % 2][:W, :], sbk_s[:, d], s1_s[:, d % 2],
                    start=True, stop=False,
                )
                tensor.wait_ge(a2t, d + 1)
                tensor.matmul(
                    pts[d % 2][:W, :], sbk_s[:, d], sp_s[:, d % 2],
                    start=False, stop=True,
                ).then_inc(p2v, 1)

    return nc


def _fast_path_ok(emissions, transitions, layer_ids, succ):
    if emissions.shape != (BATCH, C, N_LABELS):
        return False
    if transitions.shape != (N_LABELS, N_LABELS, C, C):
        return False
    if layer_ids.shape != (DEPTH, WIDTH) or succ.shape != (DEPTH, WIDTH, N_LABELS):
        return False
    if not np.array_equal(np.sort(layer_ids.reshape(-1)), np.arange(N_LABELS)):
        return False
    nz = succ != 0
    if nz.sum(axis=-1).max(initial=0) > 1:
        return False
    if nz[DEPTH - 1].any():
        return False
    if not ((succ == 0) | (succ == 1)).all():
        return False
    for d in range(DEPTH - 1):
        in_block = nz[d][:, layer_ids[d + 1]].sum()
        if in_block != nz[d].sum():
            return False
    return True


def _numpy_fallback(emissions, transitions, layer_ids, succ):
    messages = np.zeros_like(emissions)
    for d in range(layer_ids.shape[0]):
        layer = layer_ids[d]
        S = succ[d]
        x = emissions[:, :, layer] + messages[:, :, layer]          # [B,C,W]
        t = np.transpose(transitions[:, layer], (0, 2, 3, 1))       # [N,K,C,W]
        z = x[None, :, None, :, :] + t[:, None, :, :, :]            # [N,B,K,C,W]
        m = z.max(axis=3, keepdims=True)
        elem = np.squeeze(m, 3) + np.log(np.exp(z - m).sum(axis=3))
        messages = messages + np.einsum("nbkw,wn->bkn", elem, S)
    return messages


def kernel(emissions, transitions, layer_ids, succ):
    from concourse.bass_utils import run_bass_kernel_spmd

    emissions = np.ascontiguousarray(np.asarray(emissions, dtype=np.float32))
    transitions = np.ascontiguousarray(np.asarray(transitions, dtype=np.float32))
    layer_ids = np.asarray(layer_ids).astype(np.int64)
    succ = np.ascontiguousarray(np.asarray(succ, dtype=np.float32))

    if not _fast_path_ok(emissions, transitions, layer_ids, succ):
        return _numpy_fallback(emissions, transitions, layer_ids, succ)

    nz = succ != 0
    tgt = np.argmax(nz, axis=-1)  # [D, W]; 0 for empty rows (unused: S row = 0)

    # gathered 2x2 transition block per source node: tt_g[d, w, k, c]
    tt_g = transitions[tgt, layer_ids]
    # tt part of ett, layout [w, d, (c, k, b)] (b-replicated)
    ttp = np.broadcast_to(
        tt_g[:D1].transpose(1, 0, 3, 2)[..., None], (WIDTH, D1, C, C, B)
    ).reshape(WIDTH, D1, 32)

    # successor block restricted to the next layer's labels, fp16 (0/1 exact)
    sbk = np.empty((WIDTH, D1, WIDTH), dtype=np.float16)
    for d in range(D1):
        sbk[:, d, :] = succ[d][:, layer_ids[d + 1]]

    em_sh = emissions.reshape(N_CORES, B, C, N_LABELS)
    in_maps = []
    for i in range(N_CORES):
        g = em_sh[i][:, :, layer_ids[:D1]]                  # [b, c, d, w]
        emp = np.broadcast_to(
            g.transpose(3, 2, 1, 0)[:, :, :, None, :], (WIDTH, D1, C, C, B)
        ).reshape(WIDTH, D1, 32)                            # [w, d, (c,k,b)]
        ett = np.ascontiguousarray(
            np.concatenate([emp, ttp], axis=2), dtype=np.float32
        )
        in_maps.append({"ett": ett, "sbk": sbk})

    if "nc" not in _BASS_CACHE:
        _BASS_CACHE["nc"] = _build_bass()
    res = run_bass_kernel_spmd(
        _BASS_CACHE["nc"], in_maps, core_ids=list(range(N_CORES))
    )

    out = np.zeros((BATCH, C, N_LABELS), dtype=np.float32)
    for i in range(N_CORES):
        m = res.results[i]["out"].reshape(WIDTH, D1, C, B)
        blk = np.zeros((B, C, N_LABELS), dtype=np.float32)
        blk[:, :, layer_ids[1:]] = m.transpose(3, 2, 1, 0)  # [b, c, j, w]
        out[i * B : (i + 1) * B] = blk
    return out
